# revision 14
# baseline (speedup 1.0000x reference)
"""ConvCheb (K=3) Trainium2 kernel: batch-parallel across 8 cores, v2.

Per core c (batch c), slab x = inputs[c] [V, F=64]:
  y1 = L @ x          (pass 1: host pre-gathered bf16 slots, streamed S)
  z  = L @ y1         (pass 2: device dma_gather of bf16 row-pairs, streamed S)
  out = x@(W0-W2) + y1@W1 + 2*z@W2 + bias

Key differences from v1:
  - Scatter matrices S (one-hot rows scaled by lap_vals) are precomputed on
    the host and STREAMED from HBM in bf16 instead of built per-chunk on DVE
    (tensor_scalar with vector scalars measured 1185ns/chunk -> DVE 93% busy).
  - All matmul data is bf16 (fp32 PSUM accumulation).
  - Dest blocks are 64 rows wide -> S is [128, 64] (16KB bf16) per chunk.
  - Pass-2 gathers bf16 row PAIRS (256B elements, idx = col>>1 fits int16)
    from a plain [V, 64] bf16 y1; chunks are sorted by (dest block, col
    parity) so each chunk reads one 64-wide plane of the gathered pair.
  - Gathers round-robin across 4 SWDGE queues (num_swdge_queues=4) so
    descriptor generation uses all 4 Q7 core-pairs concurrently.
"""
import sys
for _p in ("/opt/trn_rl_repo",):
    if _p not in sys.path:
        sys.path.append(_p)
import numpy as np
import ml_dtypes
import concourse.bass as bass
import concourse.bacc as bacc
import concourse.mybir as mybir
import concourse.tile as tile

dt = mybir.dt
F32 = dt.float32
BF16 = dt.bfloat16
NPBF16 = ml_dtypes.bfloat16
FP8 = dt.float8e4
NPFP8 = ml_dtypes.float8_e4m3

V = 49152
F = 64
W = 64            # dest-block width (rows per psum tile)
NB = V // W       # 768 dest blocks
VH = V // 2
CP = 32           # chunks per streamed piece
NQ = 4            # SWDGE queues: 4 Q7 core-pairs generate gather descriptors concurrently
import os
PHASES = os.environ.get("KPHASES", "12")   # '1' = pass-1 only (debug bisect)
GATHER = os.environ.get("KGATHER", "1") == "1"  # 0: replace gather with dma
GB = 8            # blocks64 staged per y1 DMA
GG = 8            # block128 groups staged per out DMA


def _pad128(n):
    return max((n + 127) & ~127, 128)


def build_plan(rows, cols, vals, split_parity):
    """Chunk tables: slots sorted by dest block64 (and col parity when
    split_parity), padded per group to 128-multiples (>=128 per block).

    Returns dict with per-slot arrays (cols, local row, val) and per-block
    chunk counts. For split_parity, each block's chunks are even-cols chunks
    followed by odd-cols chunks (counts in nch_e / nch_o)."""
    order = np.argsort(rows, kind="stable")
    rows_s, cols_s, vals_s = rows[order], cols[order], vals[order]
    blk = rows_s // W
    out_cols, out_radj, out_val, parities = [], [], [], []
    nch_e = np.zeros(NB, np.int32)
    nch_o = np.zeros(NB, np.int32)
    lo_all = np.searchsorted(blk, np.arange(NB), "left")
    hi_all = np.searchsorted(blk, np.arange(NB), "right")
    for b in range(NB):
        lo, hi = lo_all[b], hi_all[b]
        rb, cb, vb = rows_s[lo:hi] - b * W, cols_s[lo:hi], vals_s[lo:hi]
        groups = []
        if split_parity:
            sel = (cb & 1) == 0
            groups.append((rb[sel], cb[sel], vb[sel], 0))
            groups.append((rb[~sel], cb[~sel], vb[~sel], 1))
        else:
            groups.append((rb, cb, vb, 0))
        for gr, gc, gv, par in groups:
            n = _pad128(len(gr))
            pad = n - len(gr)
            out_radj.append(np.concatenate([gr, np.zeros(pad, gr.dtype)]))
            out_cols.append(np.concatenate([gc, np.full(pad, par, gc.dtype)]))
            out_val.append(np.concatenate([gv, np.zeros(pad, np.float32)]))
            parities.append(np.full(n // 128, par, np.int32))
            if par == 0:
                nch_e[b] = n // 128
            else:
                nch_o[b] = n // 128
    return dict(
        cols=np.concatenate(out_cols),
        radj=np.concatenate(out_radj).astype(np.int32),
        val=np.concatenate(out_val).astype(np.float32),
        nch_e=nch_e, nch_o=nch_o,
        parity=np.concatenate(parities),
    )


def slots_to_chunk_layout(arr):
    """[nslots(, d)] -> [128, nchunks(, d)]: slot j -> [j%128, j//128]."""
    n = arr.shape[0] // 128
    a = arr.reshape(n, 128, *arr.shape[1:])
    return np.ascontiguousarray(np.moveaxis(a, 1, 0))


def wrap_idx16(idx):
    """dma_gather idx layout [128, n/16] int16: idx j at [j%16, j//16],
    replicated across the 8 groups of 16 partitions."""
    n = len(idx)
    assert n % 128 == 0
    w = np.zeros((16, n // 16), np.int16)
    for p in range(16):
        w[p, :] = idx[p::16]
    return np.ascontiguousarray(np.tile(w, (8, 1)))


def make_s(plan):
    """Dense scatter matrices [128, C, W] bf16: S[s, c, r] = val at slot
    (c*128+s) if its local dest row == r else 0."""
    nslots = len(plan["val"])
    C = nslots // 128
    s = np.zeros((C, 128, W), np.float32)
    ci = np.arange(nslots) // 128
    si = np.arange(nslots) % 128
    s[ci, si, plan["radj"]] = plan["val"]
    return np.ascontiguousarray(np.moveaxis(s, 1, 0)).astype(NPBF16)


def build_kernel(nc, C1, C2, n1, n2e, n2o, par2):
    NB128 = V // 128
    g1_d = nc.dram_tensor("g1", [128, C1, F], BF16, kind="ExternalInput")
    s1_d = nc.dram_tensor("s1", [128, C1, W], BF16, kind="ExternalInput")
    s2_d = nc.dram_tensor("s2", [128, C2, W], BF16, kind="ExternalInput")
    idx2_d = nc.dram_tensor("idx2", [128, C2 * 8], dt.int16, kind="ExternalInput")
    x0t_d = nc.dram_tensor("x0t", [F, V], BF16, kind="ExternalInput")
    w_d = nc.dram_tensor("w3", [2 * F, 3, F], BF16, kind="ExternalInput")
    bias_d = nc.dram_tensor("bias", [F, 1], F32, kind="ExternalInput")
    ident_d = nc.dram_tensor("ident", [128, 64], F32, kind="ExternalInput")
    y1_d = nc.dram_tensor("y1", [V, F], BF16)  # internal, row-major
    outt_d = nc.dram_tensor("outt", [F, V], F32, kind="ExternalOutput")

    def pieces(C):
        return [(p * CP, min(CP, C - p * CP)) for p in range((C + CP - 1) // CP)]

    with tile.TileContext(nc) as tc:
        with (
            tc.tile_pool(name="const", bufs=1) as cpool,
            tc.tile_pool(name="ybig", bufs=1) as ypool,
            tc.tile_pool(name="g1p", bufs=3) as g1pool,
            tc.tile_pool(name="s1p", bufs=3) as s1pool,
            tc.tile_pool(name="g2p", bufs=4) as g2pool,
            tc.tile_pool(name="s2p", bufs=4) as s2pool,
            tc.tile_pool(name="idxp", bufs=4) as idxpool,
            tc.tile_pool(name="ztp", bufs=2) as zpool,
            tc.tile_pool(name="xtp", bufs=2) as xtpool,
            tc.tile_pool(name="stg", bufs=3) as stgpool,
            tc.tile_pool(name="t32", bufs=3) as t32pool,
            tc.tile_pool(name="psA", bufs=2, space="PSUM") as psA,
            tc.tile_pool(name="psT", bufs=2, space="PSUM") as psT,
            tc.tile_pool(name="psB", bufs=2, space="PSUM") as psB,
            tc.tile_pool(name="psG", bufs=1, space="PSUM") as psG,
        ):
            ident_t = cpool.tile([128, 64], F32)
            nc.sync.dma_start(ident_t[:], ident_d.ap())
            w_t = cpool.tile([2 * F, 3, F], BF16)
            nc.sync.dma_start(w_t[:], w_d.ap())
            bias_t = cpool.tile([F, 1], F32)
            nc.sync.dma_start(bias_t[:], bias_d.ap())

            y1t_t = ypool.tile([128, VH], BF16)  # p = f + 64*(v >= VH)

            # ---------- PASS 1 ----------
            g1_tiles, s1_tiles = [], []
            p1list = pieces(C1)

            def emit_p1(p):
                c0, w = p1list[p]
                g1_t = g1pool.tile([128, CP, F], BF16, tag="g1")
                nc.sync.dma_start(g1_t[:, 0:w, :], g1_d.ap()[:, c0:c0 + w, :])
                s1_t = s1pool.tile([128, CP, W], BF16, tag="s1")
                nc.sync.dma_start(s1_t[:, 0:w, :], s1_d.ap()[:, c0:c0 + w, :])
                g1_tiles.append(g1_t)
                s1_tiles.append(s1_t)

            cglob = 0
            ystage = None
            for b in range(NB):
                ps = psA.tile([64, W], F32, tag="psA")
                for j in range(n1[b]):
                    while cglob // CP >= len(g1_tiles):
                        emit_p1(len(g1_tiles))
                    p, cip = divmod(cglob, CP)
                    nc.tensor.matmul(ps[:], g1_tiles[p][:, cip, :],
                                     s1_tiles[p][:, cip, :],
                                     start=(j == 0), stop=(j == n1[b] - 1))
                    cglob += 1
                half, off = divmod(b * W, VH)
                ysl = y1t_t[64 * half:64 * half + 64, off:off + W]
                nc.vector.tensor_copy(ysl, ps[:])
                t32 = t32pool.tile([64, W], F32, tag="t32")
                nc.scalar.copy(t32[:], ps[:])
                pt = psT.tile([W, 64], F32, tag="psT")
                nc.tensor.transpose(pt[:], t32[:], ident_t[0:64, :])
                if b % GB == 0:
                    ystage = stgpool.tile([W, GB, 64], BF16, tag="yst")
                nc.scalar.copy(ystage[:, b % GB, :], pt[:])
                if b % GB == GB - 1:
                    g = b // GB
                    dst = y1_d.ap().rearrange("(g e p) f -> g p e f", e=GB, p=W)
                    nc.sync.dma_start(dst[g], ystage[:])

            # ---------- PHASE boundary (debug bisect) ----------
            if '2' not in PHASES:
                ot = outt_d.ap().rearrange("f (h v) -> h f v", h=2)
                for g in range(VH // 512):
                    st = stgpool.tile([128, 512], F32, tag="dbg")
                    nc.vector.tensor_copy(st[:], y1t_t[:, g * 512:(g + 1) * 512])
                    nc.sync.dma_start(ot[0][:, g * 512:(g + 1) * 512], st[0:64, :])
                    nc.sync.dma_start(ot[1][:, g * 512:(g + 1) * 512], st[64:128, :])
                return

            # ---------- PASS 2 ----------
            y1pair = y1_d.ap().rearrange("(p two) f -> p (two f)", two=2)
            p2list = pieces(C2)
            g2_tiles, s2_tiles = [], []
            nfull_reg = nc.gpsimd.to_reg(CP * 128)

            def emit_p2(p):
                c0, w = p2list[p]
                it = idxpool.tile([128, CP * 8], dt.int16, tag="idx2")
                nc.sync.dma_start(it[:, 0:w * 8],
                                  idx2_d.ap()[:, c0 * 8:(c0 + w) * 8])
                s2_t = s2pool.tile([128, CP, W], BF16, tag="s2")
                nc.sync.dma_start(s2_t[:, 0:w, :], s2_d.ap()[:, c0:c0 + w, :])
                gt = g2pool.tile([128, CP, 2 * F], BF16, tag="g2")
                if not GATHER:
                    nc.sync.dma_start(
                        gt[:, 0:w, :],
                        y1pair.rearrange("(a p) f -> p a f", p=128)[:, 0:w, :])
                    g2_tiles.append(gt)
                    s2_tiles.append(s2_t)
                    return
                nc.gpsimd.dma_gather(
                    gt[:, 0:w, :], y1pair, it[:, 0:w * 8],
                    num_idxs=w * 128,
                    num_idxs_reg=nfull_reg if w == CP else w * 128,
                    elem_size=2 * F, single_packet=False,
                    queue_num=p % NQ,
                )
                g2_tiles.append(gt)
                s2_tiles.append(s2_t)

            cglob = 0
            ostage = None
            xt_t = None
            zt_t = None
            for b in range(NB):
                ps2 = psB.tile([64, W], F32, tag="psB")
                tot = n2e[b] + n2o[b]
                for j in range(tot):
                    while cglob // CP >= len(g2_tiles):
                        emit_p2(len(g2_tiles))
                    p, cip = divmod(cglob, CP)
                    par = par2[cglob]
                    nc.tensor.matmul(
                        ps2[:], g2_tiles[p][:, cip, 64 * par:64 * par + 64],
                        s2_tiles[p][:, cip, :],
                        start=(j == 0), stop=(j == tot - 1))
                    cglob += 1
                if b % 2 == 0:
                    zt_t = zpool.tile([64, 2, W], BF16, tag="zt")
                nc.vector.tensor_copy(zt_t[:, b % 2, :], ps2[:])
                if b % 2 == 1:
                    bb = b // 2  # block128 index
                    half = (bb * 128) // VH
                    off = (bb * 128) % VH
                    if bb % GG == 0:
                        xt_t = xtpool.tile([F, GG, 128], BF16, tag="xt")
                        nc.sync.dma_start(
                            xt_t[:], x0t_d.ap()[:, bb * 128:(bb + GG) * 128]
                            .rearrange("f (e v) -> f e v", e=GG))
                        ostage = stgpool.tile([F, GG, 128], F32, tag="ost")
                    pg = psG.tile([64, 128], F32, tag="psG")
                    nc.tensor.matmul(pg[:], w_t[0:64, 0, :],
                                     xt_t[:, bb % GG, :], start=True, stop=False)
                    nc.tensor.matmul(pg[:], w_t[0:64, 2, :],
                                     zt_t[:].rearrange("f two w -> f (two w)"),
                                     start=False, stop=True)
                    pg2 = psG.tile([64, 128], F32, tag="psG2")
                    nc.tensor.matmul(pg2[:], w_t[64 * half:64 * half + 64, 1, :],
                                     y1t_t[64 * half:64 * half + 64, off:off + 128],
                                     start=True, stop=True)
                    nc.scalar.activation(
                        ostage[:, bb % GG, :], pg[:],
                        mybir.ActivationFunctionType.Identity, bias=bias_t[:])
                    nc.vector.tensor_tensor(
                        ostage[:, bb % GG, :], ostage[:, bb % GG, :], pg2[:],
                        op=mybir.AluOpType.add)
                    if bb % GG == GG - 1:
                        g = bb // GG
                        nc.sync.dma_start(
                            outt_d.ap()[:, g * GG * 128:(g + 1) * GG * 128],
                            ostage[:].rearrange("f e v -> f (e v)"))


def host_prepare(rows, cols, vals):
    """Graph-dependent (core-independent) host tables."""
    plan1 = build_plan(rows, cols, vals, split_parity=False)
    plan2 = build_plan(rows, cols, vals, split_parity=True)
    C1 = len(plan1["val"]) // 128
    C2 = len(plan2["val"]) // 128
    tabs = dict(
        C1=C1, C2=C2,
        n1=plan1["nch_e"], n2e=plan2["nch_e"], n2o=plan2["nch_o"],
        par2=plan2["parity"],
        s1=make_s(plan1), s2=make_s(plan2),
        idx2=wrap_idx16((plan2["cols"] >> 1).astype(np.int16)),
        cols1=plan1["cols"],
        ident=np.tile(np.eye(64, dtype=np.float32), (2, 1)),
    )
    return tabs


def make_in_map(tabs, x_slab, weight, bias):
    """Per-core inputs. x_slab [V, F] f32."""
    g1 = slots_to_chunk_layout(x_slab[tabs["cols1"], :].astype(NPBF16))
    wa = weight[:, 0, :] - weight[:, 2, :]
    wb = weight[:, 1, :]
    wc = 2.0 * weight[:, 2, :]
    w3 = np.ascontiguousarray(np.tile(
        np.stack([wa, wb, wc], axis=1), (2, 1, 1))).astype(NPBF16)
    return {
        "g1": np.ascontiguousarray(g1),
        "s1": tabs["s1"], "s2": tabs["s2"], "idx2": tabs["idx2"],
        "x0t": np.ascontiguousarray(x_slab.T).astype(NPBF16),
        "w3": w3,
        "bias": bias.reshape(F, 1).astype(np.float32),
        "ident": tabs["ident"],
    }


_KERNEL_CACHE = {}


def _get_compiled(tabs):
    key = "k"
    if key not in _KERNEL_CACHE:
        nc = bacc.Bacc("TRN2", target_bir_lowering=False, debug=False,
                       num_devices=8, num_swdge_queues=NQ)
        build_kernel(nc, tabs["C1"], tabs["C2"],
                     tabs["n1"], tabs["n2e"], tabs["n2o"], tabs["par2"])
        nc.compile()
        _KERNEL_CACHE[key] = nc
    return _KERNEL_CACHE[key]


def kernel(inputs, weight, bias, lap_rows, lap_cols, lap_vals):
    from concourse.bass_utils import run_bass_kernel_spmd

    B, Vi, Fi = inputs.shape
    assert (B, Vi, Fi) == (8, V, F)
    rows = np.asarray(lap_rows).astype(np.int64)
    cols = np.asarray(lap_cols).astype(np.int64)
    vals = np.asarray(lap_vals).astype(np.float32)
    inputs = np.asarray(inputs, dtype=np.float32)
    weight = np.asarray(weight, dtype=np.float32)
    bias = np.asarray(bias, dtype=np.float32)

    tabs = host_prepare(rows, cols, vals)
    nc = _get_compiled(tabs)

    in_maps = [make_in_map(tabs, inputs[c], weight, bias) for c in range(B)]
    res = run_bass_kernel_spmd(nc, in_maps, list(range(B)))
    out = np.stack([np.ascontiguousarray(res.results[c]["outt"].T)
                    for c in range(B)])
    return out.astype(np.float32)
